# revision 1
# baseline (speedup 1.0000x reference)
"""Trainium2 Bass kernel for DigitConvolutionalModel (conv3x3 -> fc 676x128 -> relu -> fc 128x10).

Strategy
--------
The 3x3 valid conv with a (replicated) 3x3 weight is a linear map, so
    conv(x).reshape(B, 676) @ w1  ==  x @ W1eff,
where W1eff[784, 128] is assembled on the host from conv_w and w1 (68 MFLOP,
negligible). The device work is then a fused 2-layer MLP:
    out = relu(x @ W1eff + b1) @ w2 + b2.

Sharding: pure data parallel over 8 NeuronCores, 2048 batch rows per core.
The host hands each core x^T (pixels-major) so the contraction dim sits on
SBUF partitions and every DMA is fully contiguous; no on-device transposes.
Each core returns out^T [10, 2048]; the host transposes/concats.

Per-core device pipeline (batch in 512-wide blocks to fit one PSUM bank):
    fc1: 7 accumulating matmuls (6x K=128 + 1x K=16) -> PSUM [128f, 512b]
    relu+b1 on ScalarE (PSUM -> SBUF, per-partition bias)
    fc2: 1 matmul (K=128, M=10)  -> PSUM [10, 512]
    +b2 on ScalarE, DMA out.
"""

import os
import sys

import numpy as np

_TRN_REPO = "/opt/trn_rl_repo"
if _TRN_REPO not in sys.path:
    sys.path.insert(0, _TRN_REPO)

import concourse.bass as bass  # noqa: E402
import concourse.bacc as bacc  # noqa: E402
import concourse.mybir as mybir  # noqa: E402
import concourse.tile as tile  # noqa: E402
from concourse.bass_utils import run_bass_kernel_spmd  # noqa: E402

N_CORES = 8
B = 16384
BC = B // N_CORES  # 2048 batch rows per core
NPIX = 784  # 28*28 input pixels
C6 = 6  # full 128-row contraction chunks
KT = NPIX - C6 * 128  # 16-row tail chunk
NF1 = 128
NF2 = 10
NBLK = 512  # batch block = one PSUM bank of fp32
NB = BC // NBLK

_DT_NAME = os.environ.get("DIGIT_DT", "float16")
DT = getattr(mybir.dt, _DT_NAME)
DT_NP = mybir.dt.np(DT)

_NC_CACHE = None


def _build_nc():
    nc = bacc.Bacc(
        "TRN2", target_bir_lowering=False, debug=False, num_devices=N_CORES
    )
    xt = nc.dram_tensor("xt", [NPIX, BC], DT, kind="ExternalInput").ap()
    w1e = nc.dram_tensor("w1e", [NPIX, NF1], DT, kind="ExternalInput").ap()
    w2 = nc.dram_tensor("w2", [NF1, NF2], DT, kind="ExternalInput").ap()
    b1 = nc.dram_tensor("b1", [NF1, 1], mybir.dt.float32, kind="ExternalInput").ap()
    b2 = nc.dram_tensor("b2", [NF2, 1], mybir.dt.float32, kind="ExternalInput").ap()
    outT = nc.dram_tensor(
        "outT", [NF2, BC], mybir.dt.float32, kind="ExternalOutput"
    ).ap()

    with tile.TileContext(nc) as tc:
        with (
            tc.tile_pool(name="w", bufs=1) as wpool,
            tc.tile_pool(name="xin", bufs=3) as xpool,
            tc.tile_pool(name="h", bufs=2) as hpool,
            tc.tile_pool(name="o", bufs=2) as opool,
            tc.tile_pool(name="ps1", bufs=2, space=bass.MemorySpace.PSUM) as ps1pool,
            tc.tile_pool(name="ps2", bufs=2, space=bass.MemorySpace.PSUM) as ps2pool,
        ):
            # Replicated weights, loaded once.
            w1big = wpool.tile([128, C6, NF1], DT)
            nc.sync.dma_start(
                w1big[:], w1e[0 : C6 * 128, :].rearrange("(c p) f -> p c f", p=128)
            )
            w1tail = wpool.tile([KT, NF1], DT)
            nc.sync.dma_start(w1tail[:], w1e[C6 * 128 : NPIX, :])
            w2sb = wpool.tile([NF1, NF2], DT)
            nc.sync.dma_start(w2sb[:], w2[:])
            b1sb = wpool.tile([NF1, 1], mybir.dt.float32)
            nc.sync.dma_start(b1sb[:], b1[:])
            b2sb = wpool.tile([NF2, 1], mybir.dt.float32)
            nc.sync.dma_start(b2sb[:], b2[:])

            for nb in range(NB):
                bs = bass.ts(nb, NBLK)
                xbig = xpool.tile([128, C6, NBLK], DT, tag="xbig")
                nc.sync.dma_start(
                    xbig[:],
                    xt[0 : C6 * 128, bs].rearrange("(c p) n -> p c n", p=128),
                )
                xtail = xpool.tile([KT, NBLK], DT, tag="xtail")
                nc.sync.dma_start(xtail[:], xt[C6 * 128 : NPIX, bs])

                ps1 = ps1pool.tile([NF1, NBLK], mybir.dt.float32)
                for c in range(C6):
                    nc.tensor.matmul(
                        ps1[:],
                        w1big[:, c, :],
                        xbig[:, c, :],
                        start=(c == 0),
                        stop=False,
                    )
                nc.tensor.matmul(ps1[:], w1tail[:], xtail[:], start=False, stop=True)

                hT = hpool.tile([NF1, NBLK], DT)
                nc.scalar.activation(
                    hT[:], ps1[:], mybir.ActivationFunctionType.Relu, bias=b1sb[:]
                )

                ps2 = ps2pool.tile([NF2, NBLK], mybir.dt.float32)
                nc.tensor.matmul(ps2[:], w2sb[:], hT[:], start=True, stop=True)

                osb = opool.tile([NF2, NBLK], mybir.dt.float32)
                nc.scalar.activation(
                    osb[:], ps2[:], mybir.ActivationFunctionType.Identity, bias=b2sb[:]
                )
                nc.sync.dma_start(outT[:, bs], osb[:])

    nc.compile()
    return nc


def get_nc():
    global _NC_CACHE
    if _NC_CACHE is None:
        _NC_CACHE = _build_nc()
    return _NC_CACHE


def _w1eff(conv_w: np.ndarray, w1: np.ndarray) -> np.ndarray:
    """Fold the 3x3 conv into the fc1 weight: [784, 128] = C @ w1."""
    w1r = np.asarray(w1, np.float32).reshape(26, 26, NF1)
    cw = np.asarray(conv_w, np.float32)
    out = np.zeros((28, 28, NF1), np.float32)
    for di in range(3):
        for dj in range(3):
            out[di : di + 26, dj : dj + 26] += cw[di, dj] * w1r
    return out.reshape(NPIX, NF1)


def make_in_maps(x, conv_w, w1, b1, w2, b2):
    x = np.asarray(x, np.float32)
    w1e = _w1eff(conv_w, w1).astype(DT_NP)
    w2c = np.asarray(w2, np.float32).astype(DT_NP)
    b1c = np.asarray(b1, np.float32).reshape(NF1, 1)
    b2c = np.asarray(b2, np.float32).reshape(NF2, 1)
    xT = np.ascontiguousarray(x.T).astype(DT_NP)  # [784, B]
    in_maps = []
    for i in range(N_CORES):
        in_maps.append(
            {
                "xt": np.ascontiguousarray(xT[:, i * BC : (i + 1) * BC]),
                "w1e": w1e,
                "w2": w2c,
                "b1": b1c,
                "b2": b2c,
            }
        )
    return in_maps


def gather_out(results) -> np.ndarray:
    return np.concatenate([np.asarray(r["outT"]).T for r in results], axis=0)


def kernel(x, conv_w, w1, b1, w2, b2) -> np.ndarray:
    nc = get_nc()
    in_maps = make_in_maps(x, conv_w, w1, b1, w2, b2)
    res = run_bass_kernel_spmd(nc, in_maps, list(range(N_CORES)))
    return gather_out(res.results)


# revision 3
# speedup vs baseline: 1.1021x; 1.1021x over previous
"""Trainium2 Bass kernel for DigitConvolutionalModel (conv3x3 -> fc 676x128 -> relu -> fc 128x10).

Strategy
--------
The 3x3 valid conv with a replicated 3x3 weight is a linear map, so
    conv(x).reshape(B, 676) @ w1  ==  x @ W1eff,
where W1eff[784, 128] is assembled on the host from conv_w and w1 (68 MFLOP,
negligible). The device work is then a fused 2-layer MLP:
    out = relu(x @ W1eff + b1) @ w2 + b2.

Sharding: pure data parallel over 8 NeuronCores, 2048 batch rows per core.

Device-side layout choices (all driven by profile evidence):
 - The host pre-arranges x into the exact SBUF image each DMA writes:
   per core, xdev[nb][p][c*512+j] = x[nb*512+j, c*128+p] for the six full
   128-pixel contraction chunks, plus a separate [16, 2048] tail array for
   pixels 768..784. Every DMA is then partition-contiguous (7 KB runs), which
   cuts HWDGE descriptor-generation (issue) time and runs near line rate.
 - Weights/biases are packed into 2 DMAs (one fp16 blob, one fp32 blob) and
   issued on the Scalar engine's HWDGE queue so they don't serialize with the
   x-block DMAs on the Sync queue.
 - fc1 accumulates 7 matmuls into a PSUM bank (bufs=3 so the PE never waits
   on the activation drain); relu+b1 on ScalarE; fc2 on PE; +b2 on VectorE
   (keeps ScalarE's strict FIFO free for the next block's relu); one output
   DMA at the end.
"""

import os
import sys

import numpy as np

_TRN_REPO = "/opt/trn_rl_repo"
if _TRN_REPO not in sys.path:
    sys.path.insert(0, _TRN_REPO)

import concourse.bass as bass  # noqa: E402
import concourse.bacc as bacc  # noqa: E402
import concourse.mybir as mybir  # noqa: E402
import concourse.tile as tile  # noqa: E402
from concourse.bass_utils import run_bass_kernel_spmd  # noqa: E402

N_CORES = 8
B = 16384
BC = B // N_CORES  # 2048 batch rows per core
NPIX = 784  # 28*28 input pixels
C6 = 6  # full 128-row contraction chunks
KT = NPIX - C6 * 128  # 16-row tail chunk
NF1 = 128
NF2 = 10
NBLK = 512  # batch block = one PSUM bank of fp32
NB = BC // NBLK

# wpack free-dim layout: [c*128 : (c+1)*128] = w1 chunk c (c<6),
# [768:896] = w1 tail (first 16 partitions), [896:906] = w2.
WPACK_W = C6 * 128 + 128 + NF2

_DT_NAME = os.environ.get("DIGIT_DT", "float16")
DT = getattr(mybir.dt, _DT_NAME)
DT_NP = mybir.dt.np(DT)

_NC_CACHE = None


def _build_nc():
    nc = bacc.Bacc(
        "TRN2", target_bir_lowering=False, debug=False, num_devices=N_CORES
    )
    xdev = nc.dram_tensor("xdev", [NB, 128, C6 * NBLK], DT, kind="ExternalInput").ap()
    xtail = nc.dram_tensor("xtail", [KT, BC], DT, kind="ExternalInput").ap()
    wpack = nc.dram_tensor("wpack", [128, WPACK_W], DT, kind="ExternalInput").ap()
    bpack = nc.dram_tensor(
        "bpack", [128, 2], mybir.dt.float32, kind="ExternalInput"
    ).ap()
    outT = nc.dram_tensor(
        "outT", [NF2, BC], mybir.dt.float32, kind="ExternalOutput"
    ).ap()

    with tile.TileContext(nc) as tc:
        with (
            tc.tile_pool(name="w", bufs=1) as wpool,
            tc.tile_pool(name="xin", bufs=1) as xpool,
            tc.tile_pool(name="h", bufs=3) as hpool,
            tc.tile_pool(name="o", bufs=1) as opool,
            tc.tile_pool(name="ps1", bufs=3, space=bass.MemorySpace.PSUM) as ps1pool,
            tc.tile_pool(name="ps2", bufs=2, space=bass.MemorySpace.PSUM) as ps2pool,
        ):
            # x block 0 first so its data lands earliest on the Sync queue.
            xsb = []
            for nb in range(NB):
                if nb == 0:
                    t = xpool.tile([128, C6, NBLK], DT, tag="x0")
                    nc.sync.dma_start(t[:], xdev[0].rearrange("p (c n) -> p c n", c=C6))
                    xsb.append(t)
            xtsb = xpool.tile([KT, BC], DT, tag="xt")
            nc.sync.dma_start(xtsb[:], xtail[:])

            # weights on the Scalar HWDGE queue (parallel with Sync's x DMAs)
            wsb = wpool.tile([128, WPACK_W], DT)
            nc.scalar.dma_start(wsb[:], wpack[:])
            bsb = wpool.tile([128, 2], mybir.dt.float32)
            nc.scalar.dma_start(bsb[:], bpack[:])

            for nb in range(1, NB):
                t = xpool.tile([128, C6, NBLK], DT, tag=f"x{nb}")
                nc.sync.dma_start(t[:], xdev[nb].rearrange("p (c n) -> p c n", c=C6))
                xsb.append(t)

            osb = opool.tile([NF2, BC], mybir.dt.float32)

            for nb in range(NB):
                bs = bass.ts(nb, NBLK)
                ps1 = ps1pool.tile([NF1, NBLK], mybir.dt.float32)
                for c in range(C6):
                    nc.tensor.matmul(
                        ps1[:],
                        wsb[:, bass.ts(c, 128)],
                        xsb[nb][:, c, :],
                        start=(c == 0),
                        stop=False,
                    )
                nc.tensor.matmul(
                    ps1[:],
                    wsb[0:KT, C6 * 128 : C6 * 128 + NF1],
                    xtsb[:, bs],
                    start=False,
                    stop=True,
                )

                hT = hpool.tile([NF1, NBLK], DT)
                nc.scalar.activation(
                    hT[:],
                    ps1[:],
                    mybir.ActivationFunctionType.Relu,
                    bias=bsb[:, 0:1],
                )

                ps2 = ps2pool.tile([NF2, NBLK], mybir.dt.float32)
                nc.tensor.matmul(
                    ps2[:],
                    wsb[:, C6 * 128 + 128 : C6 * 128 + 128 + NF2],
                    hT[:],
                    start=True,
                    stop=True,
                )
                nc.vector.tensor_scalar_add(osb[:, bs], ps2[:], bsb[0:NF2, 1:2])

            nc.sync.dma_start(outT[:], osb[:])

    nc.compile()
    return nc


def get_nc():
    global _NC_CACHE
    if _NC_CACHE is None:
        _NC_CACHE = _build_nc()
    return _NC_CACHE


def _w1eff(conv_w: np.ndarray, w1: np.ndarray) -> np.ndarray:
    """Fold the 3x3 conv into the fc1 weight: [784, 128] = C @ w1."""
    w1r = np.asarray(w1, np.float32).reshape(26, 26, NF1)
    cw = np.asarray(conv_w, np.float32)
    out = np.zeros((28, 28, NF1), np.float32)
    for di in range(3):
        for dj in range(3):
            out[di : di + 26, dj : dj + 26] += cw[di, dj] * w1r
    return out.reshape(NPIX, NF1)


def make_in_maps(x, conv_w, w1, b1, w2, b2):
    x = np.asarray(x, np.float32)

    w1e = _w1eff(conv_w, w1)
    wpack = np.zeros((128, WPACK_W), np.float32)
    for c in range(C6):
        # SBUF partition p, free slot c*128+f  <-  w1e[c*128+p, f]
        wpack[:, c * 128 : (c + 1) * 128] = w1e[c * 128 : (c + 1) * 128, :]
    wpack[0:KT, C6 * 128 : C6 * 128 + NF1] = w1e[C6 * 128 :, :]
    wpack[:, C6 * 128 + 128 :] = np.asarray(w2, np.float32)
    wpack = wpack.astype(DT_NP)

    bpack = np.zeros((128, 2), np.float32)
    bpack[:, 0] = np.asarray(b1, np.float32)
    bpack[0:NF2, 1] = np.asarray(b2, np.float32)

    # xdev[core][nb][p][c*512+j] = x[core*2048 + nb*512 + j, c*128 + p]
    xr = x[:, : C6 * 128].reshape(N_CORES, NB, NBLK, C6, 128)
    xdev = np.ascontiguousarray(xr.transpose(0, 1, 4, 3, 2)).astype(DT_NP)
    xdev = xdev.reshape(N_CORES, NB, 128, C6 * NBLK)
    # xtail[core][p][b] = x[core*2048 + b, 768 + p]
    xt = x[:, C6 * 128 :].reshape(N_CORES, BC, KT)
    xtail = np.ascontiguousarray(xt.transpose(0, 2, 1)).astype(DT_NP)

    in_maps = []
    for i in range(N_CORES):
        in_maps.append(
            {
                "xdev": xdev[i],
                "xtail": xtail[i],
                "wpack": wpack,
                "bpack": bpack,
            }
        )
    return in_maps


def gather_out(results) -> np.ndarray:
    return np.concatenate([np.asarray(r["outT"]).T for r in results], axis=0)


def kernel(x, conv_w, w1, b1, w2, b2) -> np.ndarray:
    nc = get_nc()
    in_maps = make_in_maps(x, conv_w, w1, b1, w2, b2)
    res = run_bass_kernel_spmd(nc, in_maps, list(range(N_CORES)))
    return gather_out(res.results)


# revision 4
# speedup vs baseline: 1.1070x; 1.0045x over previous
"""Trainium2 Bass kernel for DigitConvolutionalModel (conv3x3 -> fc 676x128 -> relu -> fc 128x10).

Strategy
--------
The 3x3 valid conv with a replicated 3x3 weight is a linear map, so
    conv(x).reshape(B, 676) @ w1  ==  x @ W1eff,
where W1eff[784, 128] is assembled on the host from conv_w and w1 (68 MFLOP,
negligible). The device work is then a fused 2-layer MLP:
    out = relu(x @ W1eff + b1) @ w2 + b2.

Sharding: pure data parallel over 8 NeuronCores, 2048 batch rows per core.

Device-side layout choices (all driven by profile evidence):
 - The host pre-arranges x into the exact SBUF image each DMA writes:
   per core, xdev[nb][p][c*512+j] = x[nb*512+j, c*128+p] for the six full
   128-pixel contraction chunks, plus a separate [16, 2048] tail array for
   pixels 768..784. Every DMA is then partition-contiguous (7 KB runs), which
   cuts HWDGE descriptor-generation (issue) time and runs near line rate.
 - Weights/biases are packed into 2 DMAs (one fp16 blob, one fp32 blob) and
   issued on the Scalar engine's HWDGE queue so they don't serialize with the
   x-block DMAs on the Sync queue.
 - fc1 accumulates 7 matmuls into a PSUM bank (bufs=3 so the PE never waits
   on the activation drain); relu+b1 on ScalarE; fc2 on PE; +b2 on VectorE
   (keeps ScalarE's strict FIFO free for the next block's relu); one output
   DMA at the end.
"""

import os
import sys

import numpy as np

_TRN_REPO = "/opt/trn_rl_repo"
if _TRN_REPO not in sys.path:
    sys.path.insert(0, _TRN_REPO)

import concourse.bass as bass  # noqa: E402
import concourse.bacc as bacc  # noqa: E402
import concourse.mybir as mybir  # noqa: E402
import concourse.tile as tile  # noqa: E402
from concourse.bass_utils import run_bass_kernel_spmd  # noqa: E402

N_CORES = 8
B = 16384
BC = B // N_CORES  # 2048 batch rows per core
NPIX = 784  # 28*28 input pixels
C6 = 6  # full 128-row contraction chunks
KT = NPIX - C6 * 128  # 16-row tail chunk
NF1 = 128
NF2 = 10
NBLK = 512  # batch block = one PSUM bank of fp32
NB = BC // NBLK

# wpack free-dim layout: [c*128 : (c+1)*128] = w1 chunk c (c<6),
# [768:896] = w1 tail (first 16 partitions), [896:906] = w2.
WPACK_W = C6 * 128 + 128 + NF2

_DT_NAME = os.environ.get("DIGIT_DT", "float16")
DT = getattr(mybir.dt, _DT_NAME)
DT_NP = mybir.dt.np(DT)

_NC_CACHE = None


def _build_nc():
    nc = bacc.Bacc(
        "TRN2", target_bir_lowering=False, debug=False, num_devices=N_CORES
    )
    xdev = nc.dram_tensor("xdev", [NB, 128, C6 * NBLK], DT, kind="ExternalInput").ap()
    xtail = nc.dram_tensor("xtail", [KT, BC], DT, kind="ExternalInput").ap()
    wpack = nc.dram_tensor("wpack", [128, WPACK_W], DT, kind="ExternalInput").ap()
    bpack = nc.dram_tensor(
        "bpack", [128, 2], mybir.dt.float32, kind="ExternalInput"
    ).ap()
    outT = nc.dram_tensor(
        "outT", [NF2, BC], mybir.dt.float32, kind="ExternalOutput"
    ).ap()

    with tile.TileContext(nc) as tc:
        with (
            tc.tile_pool(name="w", bufs=1) as wpool,
            tc.tile_pool(name="xin", bufs=1) as xpool,
            tc.tile_pool(name="h", bufs=3) as hpool,
            tc.tile_pool(name="o", bufs=1) as opool,
            tc.tile_pool(name="ps1", bufs=4, space=bass.MemorySpace.PSUM) as ps1pool,
            tc.tile_pool(name="ps2", bufs=2, space=bass.MemorySpace.PSUM) as ps2pool,
        ):
            # x blocks back-to-back on the Sync HWDGE queue; everything the
            # early matmuls also need (weights, tail, biases) rides the
            # Scalar HWDGE queue in parallel.
            xsb = []
            for nb in range(NB):
                t = xpool.tile([128, C6, NBLK], DT, tag=f"x{nb}")
                nc.sync.dma_start(t[:], xdev[nb].rearrange("p (c n) -> p c n", c=C6))
                xsb.append(t)

            wsb = wpool.tile([128, WPACK_W], DT)
            nc.scalar.dma_start(wsb[:], wpack[:])
            xtsb = xpool.tile([KT, BC], DT, tag="xt")
            nc.scalar.dma_start(xtsb[:], xtail[:])
            bsb = wpool.tile([128, 2], mybir.dt.float32)
            nc.scalar.dma_start(bsb[:], bpack[:])

            osb = opool.tile([NF2, BC], mybir.dt.float32)

            # Compute blocks: the last 512-batch chunk is split in two so the
            # strictly-serial tail (relu -> fc2 -> +b2 -> store) runs on 256
            # columns instead of 512.
            starts = [0, 512, 1024, 1536, 1792]
            widths = [512, 512, 512, 256, 256]
            for bn, (s0, w) in enumerate(zip(starts, widths)):
                xt_idx = min(s0 // NBLK, NB - 1)
                j0 = s0 - xt_idx * NBLK
                ps1 = ps1pool.tile([NF1, w], mybir.dt.float32, tag="ps1")
                for c in range(C6):
                    nc.tensor.matmul(
                        ps1[:],
                        wsb[:, bass.ts(c, 128)],
                        xsb[xt_idx][:, c, j0 : j0 + w],
                        start=(c == 0),
                        stop=False,
                    )
                nc.tensor.matmul(
                    ps1[:],
                    wsb[0:KT, C6 * 128 : C6 * 128 + NF1],
                    xtsb[:, s0 : s0 + w],
                    start=False,
                    stop=True,
                )

                hT = hpool.tile([NF1, w], DT, tag="hT")
                nc.scalar.activation(
                    hT[:],
                    ps1[:],
                    mybir.ActivationFunctionType.Relu,
                    bias=bsb[:, 0:1],
                )

                ps2 = ps2pool.tile([NF2, w], mybir.dt.float32, tag="ps2")
                nc.tensor.matmul(
                    ps2[:],
                    wsb[:, C6 * 128 + 128 : C6 * 128 + 128 + NF2],
                    hT[:],
                    start=True,
                    stop=True,
                )
                nc.vector.tensor_scalar_add(osb[:, s0 : s0 + w], ps2[:], bsb[0:NF2, 1:2])
                if bn == 3:
                    # everything before the final 256 columns streams out early
                    nc.sync.dma_start(outT[:, 0:1792], osb[:, 0:1792])

            nc.sync.dma_start(outT[:, 1792:BC], osb[:, 1792:BC])

    nc.compile()
    return nc


def get_nc():
    global _NC_CACHE
    if _NC_CACHE is None:
        _NC_CACHE = _build_nc()
    return _NC_CACHE


def _w1eff(conv_w: np.ndarray, w1: np.ndarray) -> np.ndarray:
    """Fold the 3x3 conv into the fc1 weight: [784, 128] = C @ w1."""
    w1r = np.asarray(w1, np.float32).reshape(26, 26, NF1)
    cw = np.asarray(conv_w, np.float32)
    out = np.zeros((28, 28, NF1), np.float32)
    for di in range(3):
        for dj in range(3):
            out[di : di + 26, dj : dj + 26] += cw[di, dj] * w1r
    return out.reshape(NPIX, NF1)


def make_in_maps(x, conv_w, w1, b1, w2, b2):
    x = np.asarray(x, np.float32)

    w1e = _w1eff(conv_w, w1)
    wpack = np.zeros((128, WPACK_W), np.float32)
    for c in range(C6):
        # SBUF partition p, free slot c*128+f  <-  w1e[c*128+p, f]
        wpack[:, c * 128 : (c + 1) * 128] = w1e[c * 128 : (c + 1) * 128, :]
    wpack[0:KT, C6 * 128 : C6 * 128 + NF1] = w1e[C6 * 128 :, :]
    wpack[:, C6 * 128 + 128 :] = np.asarray(w2, np.float32)
    wpack = wpack.astype(DT_NP)

    bpack = np.zeros((128, 2), np.float32)
    bpack[:, 0] = np.asarray(b1, np.float32)
    bpack[0:NF2, 1] = np.asarray(b2, np.float32)

    # xdev[core][nb][p][c*512+j] = x[core*2048 + nb*512 + j, c*128 + p]
    xr = x[:, : C6 * 128].reshape(N_CORES, NB, NBLK, C6, 128)
    xdev = np.ascontiguousarray(xr.transpose(0, 1, 4, 3, 2)).astype(DT_NP)
    xdev = xdev.reshape(N_CORES, NB, 128, C6 * NBLK)
    # xtail[core][p][b] = x[core*2048 + b, 768 + p]
    xt = x[:, C6 * 128 :].reshape(N_CORES, BC, KT)
    xtail = np.ascontiguousarray(xt.transpose(0, 2, 1)).astype(DT_NP)

    in_maps = []
    for i in range(N_CORES):
        in_maps.append(
            {
                "xdev": xdev[i],
                "xtail": xtail[i],
                "wpack": wpack,
                "bpack": bpack,
            }
        )
    return in_maps


def gather_out(results) -> np.ndarray:
    return np.concatenate([np.asarray(r["outT"]).T for r in results], axis=0)


def kernel(x, conv_w, w1, b1, w2, b2) -> np.ndarray:
    nc = get_nc()
    in_maps = make_in_maps(x, conv_w, w1, b1, w2, b2)
    res = run_bass_kernel_spmd(nc, in_maps, list(range(N_CORES)))
    return gather_out(res.results)
